# revision 41
# baseline (speedup 1.0000x reference)
"""RWKV WKV kernel v4: PE-matmul chunked scan, two-pass, bf16 elementwise.

Math (per channel c, time t; sigma=max(w,0), lam=e^{min(w,0)}, qlam=e^{u+w}):
  Subchunks of L=32 steps. For t = tau + j (j = t mod L):
    eg_t  = exp(k_t - j*w - tau*sigma - beta),  beta = (L-1)/2*max(-w,0)
  (beta is a per-channel constant; y is invariant to per-channel rescaling
   of eg, and beta centers the within-subchunk exponent range into f32.)
  Per subchunk S: R^a_S = sum_j eg v,  R^b_S = sum_j eg
  Carries: G_{S} = lam32 * M_{S-1},  M_S = lam32*M_{S-1} + R_S   (lam32=lam^L)
  y_{tau+i} = (G^a_S + Sx_i + qlam*egv_i) / (G^b_S + Sbx_i + qlam*eg_i)
  with Sx_i = sum_{j<i} egv_j (strict-lower block-triangular matmul on PE),
  the qlam diagonal added via identity matmul, carries via pick matmuls.

Device layout: time on partitions ([128 t, 768 c] tiles), 32 tiles/core.
Pass 1: exp (Scalar) + egv (DVE) + R block-sums (PE) -> SBUF-resident eg/egv.
Level 2: PE transposes R -> 24 short DVE scans -> PE transposes back (bf16 G).
Pass 2: qlam mults (DVE), num/den assembled entirely in PSUM (PE), recip
(DVE), y mult (GpSimd/DVE alternating), bf16 DMA out.
Data-parallel over B across the 8 cores.
"""

import numpy as np
import ml_dtypes

import concourse.bacc as bacc
import concourse.bass as bass
import concourse.mybir as mybir
from concourse.bass_utils import run_bass_kernel_spmd
from concourse.tile import TileContext
from concourse.masks import make_identity

AluOp = mybir.AluOpType
AFT = mybir.ActivationFunctionType
F32 = mybir.dt.float32
BF16 = mybir.dt.bfloat16
F16 = mybir.dt.float16

B0, T0, C0 = 8, 4096, 768
NCORES = 8
P = 128
L = 32                 # subchunk length
SPT = P // L           # 4 subchunks per tile
NT = T0 // P           # 32 tiles
NS = T0 // L           # 128 subchunks
HB = 384               # psum half width (bank limit 512 f32)
NG = C0 // P           # 6 channel groups
HTILES = NT // 2       # 16 tiles per half


def _build_nc() -> bass.Bass:
    nc = bacc.Bacc()
    argT = nc.dram_tensor("argT", [T0, C0], F32, kind="ExternalInput")
    vT = nc.dram_tensor("vT", [T0, C0], BF16, kind="ExternalInput")
    qlm = nc.dram_tensor("qlm", [P, C0], BF16, kind="ExternalInput")
    trix = nc.dram_tensor("trix", [P, P], BF16, kind="ExternalInput")
    idm = nc.dram_tensor("idm", [P, P], BF16, kind="ExternalInput")
    # R-extraction lhsT: 8 per-tile-in-group variants of [128, 64], stacked
    rpa = nc.dram_tensor("rpa", [P, 512], BF16, kind="ExternalInput")
    rpb = nc.dram_tensor("rpb", [P, 512], BF16, kind="ExternalInput")
    # carry-pick lhsT: 16 per-tile variants of [128, 128], stacked
    pka = nc.dram_tensor("pka", [P, 16 * P], BF16, kind="ExternalInput")
    pkb = nc.dram_tensor("pkb", [P, 16 * P], BF16, kind="ExternalInput")
    lam32 = nc.dram_tensor("lam32", [P, NG], F32, kind="ExternalInput")
    lam16 = nc.dram_tensor("lam16", [P, NG], F32, kind="ExternalInput")
    yT = nc.dram_tensor("yT", [T0, C0], F16, kind="ExternalOutput")

    with TileContext(nc) as tc:
        with (
            tc.tile_pool(name="const", bufs=1) as cpool,
            tc.tile_pool(name="persist", bufs=1) as ppool,
            tc.tile_pool(name="work", bufs=3) as pool,
        ):
            qlmc = cpool.tile([P, C0], BF16)
            nc.sync.dma_start(qlmc[:], qlm[:])
            trixc = cpool.tile([P, P], BF16)
            nc.sync.dma_start(trixc[:], trix[:])
            idmc = cpool.tile([P, P], BF16)
            nc.sync.dma_start(idmc[:], idm[:])
            rpac = cpool.tile([P, 512], BF16)
            nc.sync.dma_start(rpac[:], rpa[:])
            rpbc = cpool.tile([P, 512], BF16)
            nc.sync.dma_start(rpbc[:], rpb[:])
            pkac = cpool.tile([P, 16 * P], BF16)
            nc.sync.dma_start(pkac[:], pka[:])
            pkbc = cpool.tile([P, 16 * P], BF16)
            nc.sync.dma_start(pkbc[:], pkb[:])
            lam32c = cpool.tile([P, NG], F32)
            nc.sync.dma_start(lam32c[:], lam32[:])
            lam16c = cpool.tile([P, NG], F32)
            nc.sync.dma_start(lam16c[:], lam16[:])
            idf = cpool.tile([P, P], F32)
            make_identity(nc, idf[:])

            egtiles = [ppool.tile([P, C0], BF16, name=f"egt_{t}")
                       for t in range(NT)]
            egvtiles = [ppool.tile([P, C0], BF16, name=f"egvt_{t}")
                        for t in range(NT)]
            # R rows: per tile 8 rows (a/b interleaved per subchunk), halves
            rab = ppool.tile([P, 2 * C0], F32)
            # transposed-back carries, bf16, rows = interleaved s2 per half
            gt = ppool.tile([P, 2 * C0], BF16)

            # level-2 state (persistent: scans chain across halves)
            gi = ppool.tile([P, NG * 256], F32, name="gi")
            mtas = [ppool.tile([P, 129], F32, name=f"mta_{g}") for g in range(NG)]
            mtbs = [ppool.tile([P, 129], F32, name=f"mtb_{g}") for g in range(NG)]
            lambs = [ppool.tile([P, 64], F32, name=f"lamb_{g}") for g in range(NG)]
            for g in range(NG):
                nc.vector.memset(lambs[g][:], 1.0)
                nc.vector.tensor_scalar_mul(lambs[g][:], lambs[g][:],
                                            lam32c[:, g : g + 1])
                nc.vector.memset(mtas[g][:, 0:1], 0.0)
                nc.vector.memset(mtbs[g][:, 0:1], 0.0)

            # PSUM pools open across the whole pipeline: 2 + 2 + 4 = 8 banks
            with (
                tc.tile_pool(name="psr", bufs=1, space="PSUM") as psr,
                tc.tile_pool(name="ps2", bufs=1, space="PSUM") as ps2,
            ):
                cur_pr = [None]

                def pass1_tile(t):
                    at = pool.tile([P, C0], F32, tag="at", bufs=3, name=f"at_{t}")
                    nc.sync.dma_start(at[:], argT[P * t : P * (t + 1), :])
                    vt = pool.tile([P, C0], BF16, tag="vt", bufs=3,
                                   name=f"vt_{t}")
                    nc.sync.dma_start(vt[:], vT[P * t : P * (t + 1), :])
                    eg = egtiles[t][:]
                    nc.scalar.activation(eg, at[:], AFT.Exp)
                    egv = egvtiles[t][:]
                    nc.vector.tensor_tensor(egv, eg, vt[:], op=AluOp.mult)
                    # R rows for groups of 8 tiles accumulate into one
                    # [64, HB] psum tile; per-tile lhsT variants write
                    # disjoint rows (accumulation adds zeros elsewhere).
                    h = t // HTILES
                    m8 = t % 8
                    grp = t // 8
                    if m8 == 0:
                        cur_pr[0] = psr.tile([P, 512], F32, tag="pr",
                                             bufs=1, name=f"pr_{grp}")
                    lcols = slice(64 * m8, 64 * (m8 + 1))
                    for hb in range(2):
                        cols = slice(hb * HB, (hb + 1) * HB)
                        out = cur_pr[0][64 * hb : 64 * (hb + 1), 0:HB]
                        nc.tensor.matmul(out, rpac[:, lcols], egv[:, cols],
                                         start=(m8 == 0), stop=False)
                    for hb in range(2):
                        cols = slice(hb * HB, (hb + 1) * HB)
                        out = cur_pr[0][64 * hb : 64 * (hb + 1), 0:HB]
                        nc.tensor.matmul(out, rpbc[:, lcols], eg[:, cols],
                                         start=False, stop=(m8 == 7))
                    if m8 == 7:
                        r0 = 64 * (grp % 2)
                        for hb in range(2):
                            nc.scalar.copy(
                                rab[r0 : r0 + 64,
                                    h * C0 + hb * HB : h * C0 + (hb + 1) * HB],
                                cur_pr[0][64 * hb : 64 * (hb + 1), 0:HB],
                            )

                def level2_half(h):
                    # R'' = lam32*R via two lam16 scalings (lam32 itself
                    # underflows f32 for strongly negative w)
                    hc = h * C0
                    for g in range(NG):
                        rt = pool.tile([P, P], F32, tag="rt", bufs=2,
                                       name=f"rt_{h}_{g}")
                        pt0 = psr.tile([P, 512], F32, tag="pr", bufs=1,
                                       name=f"pt_{h}_{g}")
                        nc.tensor.transpose(pt0[:, 0:P],
                                            rab[:, hc + g * P : hc + (g + 1) * P],
                                            idf[:])
                        nc.scalar.activation(rt[:], pt0[:, 0:P], AFT.Copy,
                                             scale=lam16c[:, g : g + 1])
                        nc.vector.tensor_scalar_mul(rt[:], rt[:],
                                                    lam16c[:, g : g + 1])
                        # chained scans: a at even cols, b at odd cols
                        mta, mtb = mtas[g], mtbs[g]
                        c0s, c1s = 1 + 64 * h, 65 + 64 * h
                        init_a = 0.0 if h == 0 else mta[:, 64:65]
                        init_b = 0.0 if h == 0 else mtb[:, 64:65]
                        nc.vector.tensor_tensor_scan(
                            mta[:, c0s : c0s + 64], lambs[g][:],
                            rt[:, 0:128:2], init_a,
                            op0=AluOp.mult, op1=AluOp.add)
                        nc.vector.tensor_tensor_scan(
                            mtb[:, c0s : c0s + 64], lambs[g][:],
                            rt[:, 1:128:2], init_b,
                            op0=AluOp.mult, op1=AluOp.add)
                        # G_S = scan state at S-1 (R pre-scaled by lam32):
                        # gi[:, 256g + 2S+type] = mt[:, S]
                        gsl = gi[:, 256 * g + 128 * h : 256 * g + 128 * (h + 1)]
                        nc.vector.tensor_copy(gsl[:, 0:128:2],
                                              mtas[g][:, 64 * h : 64 * (h + 1)])
                        nc.vector.tensor_copy(gsl[:, 1:128:2],
                                              mtbs[g][:, 64 * h : 64 * (h + 1)])
                        ptg = psr.tile([P, 512], F32, tag="pr", bufs=1,
                                       name=f"ptg_{h}_{g}")
                        nc.tensor.transpose(ptg[:, 0:P], gsl, idf[:])
                        nc.scalar.copy(gt[:, hc + g * P : hc + (g + 1) * P],
                                       ptg[:, 0:P])

                def pass2_tile(t):
                    eg = egtiles[t][:]
                    egv = egvtiles[t][:]
                    eg2 = pool.tile([P, C0], BF16, tag="eg2", bufs=2,
                                    name=f"eg2_{t}")
                    # on odd tiles DVE owns the y-mult; eg2 goes to gpsimd
                    eng2 = nc.gpsimd if t % 2 == 1 else nc.vector
                    eng2.tensor_tensor(eg2[:], eg, qlmc[:], op=AluOp.mult)
                    egvq = pool.tile([P, C0], BF16, tag="egvq", bufs=2,
                                     name=f"egvq_{t}")
                    nc.vector.tensor_tensor(egvq[:], egv, qlmc[:], op=AluOp.mult)
                    h = t // HTILES
                    m = t % HTILES
                    pcols = slice(m * P, (m + 1) * P)
                    yt = pool.tile([P, C0], F16, tag="yt", bufs=2, name=f"yt_{t}")
                    pns_ = [None, None]
                    recs = [None, None]
                    pds = [None, None]
                    for hb in range(2):
                        pnum = ps2.tile([P, 512], F32, tag=f"pn{hb}", bufs=2,
                                        name=f"pn_{t}_{hb}")
                        pns_[hb] = pnum[:, 0:HB]
                        pden = ps2.tile([P, 512], F32, tag=f"pd{hb}", bufs=1,
                                        name=f"pd_{t}_{hb}")
                        pds[hb] = pden[:, 0:HB]
                    # weight-reuse matmul order
                    for hb in range(2):
                        cols = slice(hb * HB, (hb + 1) * HB)
                        nc.tensor.matmul(pns_[hb], trixc[:], egv[:, cols],
                                         start=True, stop=False)
                        nc.tensor.matmul(pds[hb], trixc[:], eg[:, cols],
                                         start=True, stop=False)
                    for hb in range(2):
                        cols = slice(hb * HB, (hb + 1) * HB)
                        nc.tensor.matmul(pns_[hb], idmc[:], egvq[:, cols],
                                         start=False, stop=False)
                        nc.tensor.matmul(pds[hb], idmc[:], eg2[:, cols],
                                         start=False, stop=False)
                    for hb in range(2):
                        gcols = slice(h * C0 + hb * HB, h * C0 + (hb + 1) * HB)
                        nc.tensor.matmul(pns_[hb], pkac[:, pcols], gt[:, gcols],
                                         start=False, stop=True)
                    for hb in range(2):
                        gcols = slice(h * C0 + hb * HB, h * C0 + (hb + 1) * HB)
                        nc.tensor.matmul(pds[hb], pkbc[:, pcols], gt[:, gcols],
                                         start=False, stop=True)
                    for hb in range(2):
                        cols = slice(hb * HB, (hb + 1) * HB)
                        rec = pool.tile([P, HB], F32, tag=f"rec{hb}", bufs=2,
                                        name=f"rec_{t}_{hb}")
                        nc.vector.reciprocal_approx_fast(rec[:], pds[hb])
                        recs[hb] = rec
                        if (t + hb) % 2 == 0:
                            # gpsimd cannot read PSUM: scalar evacuates num
                            # to SBUF (bf16), gpsimd multiplies
                            pns2 = pool.tile([P, HB], BF16, tag=f"pns{hb}",
                                             bufs=2, name=f"pns_{t}_{hb}")
                            nc.scalar.copy(pns2[:], pns_[hb])
                            nc.gpsimd.tensor_tensor(yt[:, cols], pns2[:],
                                                    recs[hb][:], op=AluOp.mult)
                        else:
                            nc.vector.tensor_tensor(yt[:, cols], pns_[hb],
                                                    recs[hb][:], op=AluOp.mult)
                    nc.sync.dma_start(yT[P * t : P * (t + 1), :], yt[:])

                # ---- half-pipelined schedule ----
                for t in range(HTILES):
                    pass1_tile(t)
                level2_half(0)
                for i in range(HTILES):
                    pass1_tile(HTILES + i)
                    pass2_tile(i)
                level2_half(1)
                for t in range(HTILES, NT):
                    pass2_tile(t)
    nc.finalize()
    return nc


_NC_CACHE: list = []


def _get_nc() -> bass.Bass:
    if not _NC_CACHE:
        _NC_CACHE.append(_build_nc())
    return _NC_CACHE[0]


def _host_consts(w: np.ndarray, u: np.ndarray):
    w64 = w.astype(np.float64)
    u64 = u.astype(np.float64)
    sig = np.maximum(w64, 0.0)
    lam32v = np.exp(np.minimum(w64, 0.0) * L).astype(np.float32)
    lam16v = np.exp(np.minimum(w64, 0.0) * (L // 2)).astype(np.float32)
    qlam = np.exp(u64 + w64).astype(np.float32)
    beta = (L - 1) / 2.0 * np.maximum(-w64, 0.0)

    qlm = np.ascontiguousarray(
        np.broadcast_to(qlam.astype(ml_dtypes.bfloat16), (P, C0))
    )
    j = np.arange(P)
    blk = j // L
    trix = ((blk[:, None] == blk[None, :]) & (j[:, None] < j[None, :])).astype(
        ml_dtypes.bfloat16
    )
    idm = np.eye(P, dtype=ml_dtypes.bfloat16)
    # R-extraction lhsT variants: m8-th variant is [128, 64] with ones at
    # [j, 8*m8 + 2*(j//L) (+1 for b)]
    rpa = np.zeros((P, 512), dtype=ml_dtypes.bfloat16)
    rpb = np.zeros((P, 512), dtype=ml_dtypes.bfloat16)
    for m8 in range(8):
        rcol = 64 * m8 + 8 * m8 + 2 * blk
        rpa[j, rcol] = 1
        rpb[j, rcol + 1] = 1
    # carry-pick lhsT variants: m-th is [128, 128] with pka[r, i] = 1 iff
    # r == 8*m + 2*(i//L) (+1 for b)
    pka = np.zeros((P, 16 * P), dtype=ml_dtypes.bfloat16)
    pkb = np.zeros((P, 16 * P), dtype=ml_dtypes.bfloat16)
    for m in range(16):
        rrow = 8 * m + 2 * blk
        pka[rrow, m * P + j] = 1
        pkb[rrow + 1, m * P + j] = 1
    lam32col = np.ascontiguousarray(lam32v.reshape(NG, P).T)
    lam16col = np.ascontiguousarray(lam16v.reshape(NG, P).T)

    t_idx = np.arange(T0)
    jmod = (t_idx % L).astype(np.float64)
    tau = (t_idx - jmod).astype(np.float64)
    argbase = (
        -jmod[:, None] * w64[None, :]
        - tau[:, None] * sig[None, :]
        - beta[None, :]
    )
    return qlm, trix, idm, rpa, rpb, pka, pkb, lam32col, lam16col, argbase


def kernel(B, T, C, w, u, k, v):
    B, T, C = int(B), int(T), int(C)
    assert (B, T, C) == (B0, T0, C0), f"compiled for {(B0, T0, C0)}, got {(B, T, C)}"
    w = np.asarray(w, dtype=np.float32)
    u = np.asarray(u, dtype=np.float32)
    k = np.asarray(k, dtype=np.float32)
    v = np.asarray(v, dtype=np.float32)

    qlm, trix, idm, rpa, rpb, pka, pkb, lam32col, lam16col, argbase = _host_consts(
        w, u
    )
    in_maps = []
    for b in range(NCORES):
        arg_b = (k[b].astype(np.float64) + argbase).astype(np.float32)
        in_maps.append(
            {
                "argT": arg_b,
                "vT": v[b].astype(ml_dtypes.bfloat16),
                "qlm": qlm,
                "trix": trix,
                "idm": idm,
                "rpa": rpa,
                "rpb": rpb,
                "pka": pka,
                "pkb": pkb,
                "lam32": lam32col,
                "lam16": lam16col,
            }
        )

    res = run_bass_kernel_spmd(_get_nc(), in_maps, list(range(NCORES)))
    out = np.stack(
        [res.results[i]["yT"].astype(np.float32) for i in range(NCORES)], axis=0
    )
    return np.ascontiguousarray(out)


# revision 42
# speedup vs baseline: 1.0190x; 1.0190x over previous
"""RWKV WKV kernel v4: PE-matmul chunked scan, two-pass, bf16 elementwise.

Math (per channel c, time t; sigma=max(w,0), lam=e^{min(w,0)}, qlam=e^{u+w}):
  Subchunks of L=32 steps. For t = tau + j (j = t mod L):
    eg_t  = exp(k_t - j*w - tau*sigma - beta),  beta = (L-1)/2*max(-w,0)
  (beta is a per-channel constant; y is invariant to per-channel rescaling
   of eg, and beta centers the within-subchunk exponent range into f32.)
  Per subchunk S: R^a_S = sum_j eg v,  R^b_S = sum_j eg
  Carries: G_{S} = lam32 * M_{S-1},  M_S = lam32*M_{S-1} + R_S   (lam32=lam^L)
  y_{tau+i} = (G^a_S + Sx_i + qlam*egv_i) / (G^b_S + Sbx_i + qlam*eg_i)
  with Sx_i = sum_{j<i} egv_j (strict-lower block-triangular matmul on PE),
  the qlam diagonal added via identity matmul, carries via pick matmuls.

Device layout: time on partitions ([128 t, 768 c] tiles), 32 tiles/core.
Pass 1: exp (Scalar) + egv (DVE) + R block-sums (PE) -> SBUF-resident eg/egv.
Level 2: PE transposes R -> 24 short DVE scans -> PE transposes back (bf16 G).
Pass 2: qlam mults (DVE), num/den assembled entirely in PSUM (PE), recip
(DVE), y mult (GpSimd/DVE alternating), bf16 DMA out.
Data-parallel over B across the 8 cores.
"""

import numpy as np
import ml_dtypes

import concourse.bacc as bacc
import concourse.bass as bass
import concourse.mybir as mybir
from concourse.bass_utils import run_bass_kernel_spmd
from concourse.tile import TileContext
from concourse.masks import make_identity

AluOp = mybir.AluOpType
AFT = mybir.ActivationFunctionType
F32 = mybir.dt.float32
BF16 = mybir.dt.bfloat16
F16 = mybir.dt.float16

B0, T0, C0 = 8, 4096, 768
NCORES = 8
P = 128
L = 32                 # subchunk length
SPT = P // L           # 4 subchunks per tile
NT = T0 // P           # 32 tiles
NS = T0 // L           # 128 subchunks
HB = 384               # psum half width (bank limit 512 f32)
NG = C0 // P           # 6 channel groups
HTILES = NT // 2       # 16 tiles per half


def _build_nc() -> bass.Bass:
    nc = bacc.Bacc()
    argT = nc.dram_tensor("argT", [T0, C0], F32, kind="ExternalInput")
    vT = nc.dram_tensor("vT", [T0, C0], BF16, kind="ExternalInput")
    qlm = nc.dram_tensor("qlm", [P, C0], BF16, kind="ExternalInput")
    trix = nc.dram_tensor("trix", [P, P], BF16, kind="ExternalInput")
    idm = nc.dram_tensor("idm", [P, P], BF16, kind="ExternalInput")
    # R-extraction lhsT: 8 per-tile-in-group variants of [128, 64], stacked
    rpa = nc.dram_tensor("rpa", [P, 512], BF16, kind="ExternalInput")
    rpb = nc.dram_tensor("rpb", [P, 512], BF16, kind="ExternalInput")
    # carry-pick lhsT: 16 per-tile variants of [128, 128], stacked
    pka = nc.dram_tensor("pka", [P, 16 * P], BF16, kind="ExternalInput")
    pkb = nc.dram_tensor("pkb", [P, 16 * P], BF16, kind="ExternalInput")
    lam32 = nc.dram_tensor("lam32", [P, NG], F32, kind="ExternalInput")
    lam16 = nc.dram_tensor("lam16", [P, NG], F32, kind="ExternalInput")
    yT = nc.dram_tensor("yT", [T0, C0], F16, kind="ExternalOutput")

    with TileContext(nc) as tc:
        with (
            tc.tile_pool(name="const", bufs=1) as cpool,
            tc.tile_pool(name="persist", bufs=1) as ppool,
            tc.tile_pool(name="work", bufs=3) as pool,
        ):
            qlmc = cpool.tile([P, C0], BF16)
            nc.sync.dma_start(qlmc[:], qlm[:])
            trixc = cpool.tile([P, P], BF16)
            nc.sync.dma_start(trixc[:], trix[:])
            idmc = cpool.tile([P, P], BF16)
            nc.sync.dma_start(idmc[:], idm[:])
            rpac = cpool.tile([P, 512], BF16)
            nc.sync.dma_start(rpac[:], rpa[:])
            rpbc = cpool.tile([P, 512], BF16)
            nc.sync.dma_start(rpbc[:], rpb[:])
            pkac = cpool.tile([P, 16 * P], BF16)
            nc.sync.dma_start(pkac[:], pka[:])
            pkbc = cpool.tile([P, 16 * P], BF16)
            nc.sync.dma_start(pkbc[:], pkb[:])
            lam32c = cpool.tile([P, NG], F32)
            nc.sync.dma_start(lam32c[:], lam32[:])
            lam16c = cpool.tile([P, NG], F32)
            nc.sync.dma_start(lam16c[:], lam16[:])
            idf = cpool.tile([P, P], F32)
            make_identity(nc, idf[:])

            egtiles = [ppool.tile([P, C0], BF16, name=f"egt_{t}")
                       for t in range(NT)]
            egvtiles = [ppool.tile([P, C0], BF16, name=f"egvt_{t}")
                        for t in range(NT)]
            # R rows: per tile 8 rows (a/b interleaved per subchunk), halves
            rab = ppool.tile([P, 2 * C0], F32)
            # transposed-back carries, bf16, rows = interleaved s2 per half
            gt = ppool.tile([P, 2 * C0], BF16)

            # level-2 state (persistent: scans chain across halves)
            gi = ppool.tile([P, NG * 256], F32, name="gi")
            mtas = [ppool.tile([P, 129], F32, name=f"mta_{g}") for g in range(NG)]
            mtbs = [ppool.tile([P, 129], F32, name=f"mtb_{g}") for g in range(NG)]
            lambs = [ppool.tile([P, 64], F32, name=f"lamb_{g}") for g in range(NG)]
            for g in range(NG):
                nc.vector.memset(lambs[g][:], 1.0)
                nc.vector.tensor_scalar_mul(lambs[g][:], lambs[g][:],
                                            lam32c[:, g : g + 1])
                nc.vector.memset(mtas[g][:, 0:1], 0.0)
                nc.vector.memset(mtbs[g][:, 0:1], 0.0)

            # PSUM pools open across the whole pipeline: 2 + 2 + 4 = 8 banks
            with (
                tc.tile_pool(name="psr", bufs=1, space="PSUM") as psr,
                tc.tile_pool(name="ps2", bufs=1, space="PSUM") as ps2,
            ):
                cur_pr = [None]

                def pass1_tile(t):
                    at = pool.tile([P, C0], F32, tag="at", bufs=3, name=f"at_{t}")
                    nc.sync.dma_start(at[:], argT[P * t : P * (t + 1), :])
                    vt = pool.tile([P, C0], BF16, tag="vt", bufs=3,
                                   name=f"vt_{t}")
                    nc.sync.dma_start(vt[:], vT[P * t : P * (t + 1), :])
                    eg = egtiles[t][:]
                    nc.scalar.activation(eg, at[:], AFT.Exp)
                    egv = egvtiles[t][:]
                    nc.vector.tensor_tensor(egv, eg, vt[:], op=AluOp.mult)
                    # R rows for groups of 8 tiles accumulate into one
                    # [64, HB] psum tile; per-tile lhsT variants write
                    # disjoint rows (accumulation adds zeros elsewhere).
                    h = t // HTILES
                    m8 = t % 8
                    grp = t // 8
                    if m8 == 0:
                        cur_pr[0] = psr.tile([P, 512], F32, tag="pr",
                                             bufs=1, name=f"pr_{grp}")
                    lcols = slice(64 * m8, 64 * (m8 + 1))
                    for hb in range(2):
                        cols = slice(hb * HB, (hb + 1) * HB)
                        out = cur_pr[0][64 * hb : 64 * (hb + 1), 0:HB]
                        nc.tensor.matmul(out, rpac[:, lcols], egv[:, cols],
                                         start=(m8 == 0), stop=False)
                    for hb in range(2):
                        cols = slice(hb * HB, (hb + 1) * HB)
                        out = cur_pr[0][64 * hb : 64 * (hb + 1), 0:HB]
                        nc.tensor.matmul(out, rpbc[:, lcols], eg[:, cols],
                                         start=False, stop=(m8 == 7))
                    if m8 == 7:
                        r0 = 64 * (grp % 2)
                        for hb in range(2):
                            nc.scalar.copy(
                                rab[r0 : r0 + 64,
                                    h * C0 + hb * HB : h * C0 + (hb + 1) * HB],
                                cur_pr[0][64 * hb : 64 * (hb + 1), 0:HB],
                            )

                def level2_half(h):
                    # R'' = lam32*R via two lam16 scalings (lam32 itself
                    # underflows f32 for strongly negative w)
                    hc = h * C0
                    for g in range(NG):
                        rt = pool.tile([P, P], F32, tag="rt", bufs=2,
                                       name=f"rt_{h}_{g}")
                        pt0 = psr.tile([P, 512], F32, tag="pr", bufs=1,
                                       name=f"pt_{h}_{g}")
                        nc.tensor.transpose(pt0[:, 0:P],
                                            rab[:, hc + g * P : hc + (g + 1) * P],
                                            idf[:])
                        nc.scalar.activation(rt[:], pt0[:, 0:P], AFT.Copy,
                                             scale=lam16c[:, g : g + 1])
                        nc.vector.tensor_scalar_mul(rt[:], rt[:],
                                                    lam16c[:, g : g + 1])
                        # chained scans: a at even cols, b at odd cols
                        mta, mtb = mtas[g], mtbs[g]
                        c0s, c1s = 1 + 64 * h, 65 + 64 * h
                        init_a = 0.0 if h == 0 else mta[:, 64:65]
                        init_b = 0.0 if h == 0 else mtb[:, 64:65]
                        nc.vector.tensor_tensor_scan(
                            mta[:, c0s : c0s + 64], lambs[g][:],
                            rt[:, 0:128:2], init_a,
                            op0=AluOp.mult, op1=AluOp.add)
                        nc.vector.tensor_tensor_scan(
                            mtb[:, c0s : c0s + 64], lambs[g][:],
                            rt[:, 1:128:2], init_b,
                            op0=AluOp.mult, op1=AluOp.add)
                        # G_S = scan state at S-1 (R pre-scaled by lam32):
                        # gi[:, 256g + 2S+type] = mt[:, S]
                        gsl = gi[:, 256 * g + 128 * h : 256 * g + 128 * (h + 1)]
                        nc.vector.tensor_copy(gsl[:, 0:128:2],
                                              mtas[g][:, 64 * h : 64 * (h + 1)])
                        nc.vector.tensor_copy(gsl[:, 1:128:2],
                                              mtbs[g][:, 64 * h : 64 * (h + 1)])
                        ptg = psr.tile([P, 512], F32, tag="pr", bufs=1,
                                       name=f"ptg_{h}_{g}")
                        nc.tensor.transpose(ptg[:, 0:P], gsl, idf[:])
                        nc.scalar.copy(gt[:, hc + g * P : hc + (g + 1) * P],
                                       ptg[:, 0:P])

                def pass2_tile(t):
                    eg = egtiles[t][:]
                    egv = egvtiles[t][:]
                    eg2 = pool.tile([P, C0], BF16, tag="eg2", bufs=2,
                                    name=f"eg2_{t}")
                    # on odd tiles DVE owns the y-mult; eg2 goes to gpsimd
                    eng2 = nc.gpsimd if t % 2 == 1 else nc.vector
                    eng2.tensor_tensor(eg2[:], eg, qlmc[:], op=AluOp.mult)
                    egvq = pool.tile([P, C0], BF16, tag="egvq", bufs=2,
                                     name=f"egvq_{t}")
                    nc.vector.tensor_tensor(egvq[:], egv, qlmc[:], op=AluOp.mult)
                    h = t // HTILES
                    m = t % HTILES
                    pcols = slice(m * P, (m + 1) * P)
                    yt = pool.tile([P, C0], F16, tag="yt", bufs=2, name=f"yt_{t}")
                    pns_ = [None, None]
                    recs = [None, None]
                    pds = [None, None]
                    for hb in range(2):
                        pnum = ps2.tile([P, 512], F32, tag=f"pn{hb}", bufs=2,
                                        name=f"pn_{t}_{hb}")
                        pns_[hb] = pnum[:, 0:HB]
                        pden = ps2.tile([P, 512], F32, tag=f"pd{hb}", bufs=1,
                                        name=f"pd_{t}_{hb}")
                        pds[hb] = pden[:, 0:HB]
                    # weight-reuse matmul order
                    for hb in range(2):
                        cols = slice(hb * HB, (hb + 1) * HB)
                        nc.tensor.matmul(pns_[hb], trixc[:], egv[:, cols],
                                         start=True, stop=False)
                        nc.tensor.matmul(pds[hb], trixc[:], eg[:, cols],
                                         start=True, stop=False)
                    for hb in range(2):
                        cols = slice(hb * HB, (hb + 1) * HB)
                        nc.tensor.matmul(pns_[hb], idmc[:], egvq[:, cols],
                                         start=False, stop=False)
                        nc.tensor.matmul(pds[hb], idmc[:], eg2[:, cols],
                                         start=False, stop=False)
                    for hb in range(2):
                        gcols = slice(h * C0 + hb * HB, h * C0 + (hb + 1) * HB)
                        nc.tensor.matmul(pns_[hb], pkac[:, pcols], gt[:, gcols],
                                         start=False, stop=True)
                    for hb in range(2):
                        gcols = slice(h * C0 + hb * HB, h * C0 + (hb + 1) * HB)
                        nc.tensor.matmul(pds[hb], pkbc[:, pcols], gt[:, gcols],
                                         start=False, stop=True)
                    for hb in range(2):
                        cols = slice(hb * HB, (hb + 1) * HB)
                        rec = pool.tile([P, HB], F32, tag=f"rec{hb}", bufs=2,
                                        name=f"rec_{t}_{hb}")
                        nc.vector.reciprocal_approx_fast(rec[:], pds[hb])
                        recs[hb] = rec
                        nc.vector.tensor_tensor(yt[:, cols], pns_[hb],
                                                recs[hb][:], op=AluOp.mult)
                    nc.scalar.dma_start(yT[P * t : P * (t + 1), :], yt[:])

                # ---- half-pipelined schedule ----
                for t in range(HTILES):
                    pass1_tile(t)
                level2_half(0)
                for i in range(HTILES):
                    pass1_tile(HTILES + i)
                    pass2_tile(i)
                level2_half(1)
                for t in range(HTILES, NT):
                    pass2_tile(t)
    nc.finalize()
    return nc


_NC_CACHE: list = []


def _get_nc() -> bass.Bass:
    if not _NC_CACHE:
        _NC_CACHE.append(_build_nc())
    return _NC_CACHE[0]


def _host_consts(w: np.ndarray, u: np.ndarray):
    w64 = w.astype(np.float64)
    u64 = u.astype(np.float64)
    sig = np.maximum(w64, 0.0)
    lam32v = np.exp(np.minimum(w64, 0.0) * L).astype(np.float32)
    lam16v = np.exp(np.minimum(w64, 0.0) * (L // 2)).astype(np.float32)
    qlam = np.exp(u64 + w64).astype(np.float32)
    beta = (L - 1) / 2.0 * np.maximum(-w64, 0.0)

    qlm = np.ascontiguousarray(
        np.broadcast_to(qlam.astype(ml_dtypes.bfloat16), (P, C0))
    )
    j = np.arange(P)
    blk = j // L
    trix = ((blk[:, None] == blk[None, :]) & (j[:, None] < j[None, :])).astype(
        ml_dtypes.bfloat16
    )
    idm = np.eye(P, dtype=ml_dtypes.bfloat16)
    # R-extraction lhsT variants: m8-th variant is [128, 64] with ones at
    # [j, 8*m8 + 2*(j//L) (+1 for b)]
    rpa = np.zeros((P, 512), dtype=ml_dtypes.bfloat16)
    rpb = np.zeros((P, 512), dtype=ml_dtypes.bfloat16)
    for m8 in range(8):
        rcol = 64 * m8 + 8 * m8 + 2 * blk
        rpa[j, rcol] = 1
        rpb[j, rcol + 1] = 1
    # carry-pick lhsT variants: m-th is [128, 128] with pka[r, i] = 1 iff
    # r == 8*m + 2*(i//L) (+1 for b)
    pka = np.zeros((P, 16 * P), dtype=ml_dtypes.bfloat16)
    pkb = np.zeros((P, 16 * P), dtype=ml_dtypes.bfloat16)
    for m in range(16):
        rrow = 8 * m + 2 * blk
        pka[rrow, m * P + j] = 1
        pkb[rrow + 1, m * P + j] = 1
    lam32col = np.ascontiguousarray(lam32v.reshape(NG, P).T)
    lam16col = np.ascontiguousarray(lam16v.reshape(NG, P).T)

    t_idx = np.arange(T0)
    jmod = (t_idx % L).astype(np.float64)
    tau = (t_idx - jmod).astype(np.float64)
    argbase = (
        -jmod[:, None] * w64[None, :]
        - tau[:, None] * sig[None, :]
        - beta[None, :]
    )
    return qlm, trix, idm, rpa, rpb, pka, pkb, lam32col, lam16col, argbase


def kernel(B, T, C, w, u, k, v):
    B, T, C = int(B), int(T), int(C)
    assert (B, T, C) == (B0, T0, C0), f"compiled for {(B0, T0, C0)}, got {(B, T, C)}"
    w = np.asarray(w, dtype=np.float32)
    u = np.asarray(u, dtype=np.float32)
    k = np.asarray(k, dtype=np.float32)
    v = np.asarray(v, dtype=np.float32)

    qlm, trix, idm, rpa, rpb, pka, pkb, lam32col, lam16col, argbase = _host_consts(
        w, u
    )
    in_maps = []
    for b in range(NCORES):
        arg_b = (k[b].astype(np.float64) + argbase).astype(np.float32)
        in_maps.append(
            {
                "argT": arg_b,
                "vT": v[b].astype(ml_dtypes.bfloat16),
                "qlm": qlm,
                "trix": trix,
                "idm": idm,
                "rpa": rpa,
                "rpb": rpb,
                "pka": pka,
                "pkb": pkb,
                "lam32": lam32col,
                "lam16": lam16col,
            }
        )

    res = run_bass_kernel_spmd(_get_nc(), in_maps, list(range(NCORES)))
    out = np.stack(
        [res.results[i]["yT"].astype(np.float32) for i in range(NCORES)], axis=0
    )
    return np.ascontiguousarray(out)


# revision 43
# speedup vs baseline: 1.0201x; 1.0012x over previous
"""RWKV WKV kernel v4: PE-matmul chunked scan, two-pass, bf16 elementwise.

Math (per channel c, time t; sigma=max(w,0), lam=e^{min(w,0)}, qlam=e^{u+w}):
  Subchunks of L=32 steps. For t = tau + j (j = t mod L):
    eg_t  = exp(k_t - j*w - tau*sigma - beta),  beta = (L-1)/2*max(-w,0)
  (beta is a per-channel constant; y is invariant to per-channel rescaling
   of eg, and beta centers the within-subchunk exponent range into f32.)
  Per subchunk S: R^a_S = sum_j eg v,  R^b_S = sum_j eg
  Carries: G_{S} = lam32 * M_{S-1},  M_S = lam32*M_{S-1} + R_S   (lam32=lam^L)
  y_{tau+i} = (G^a_S + Sx_i + qlam*egv_i) / (G^b_S + Sbx_i + qlam*eg_i)
  with Sx_i = sum_{j<i} egv_j (strict-lower block-triangular matmul on PE),
  the qlam diagonal added via identity matmul, carries via pick matmuls.

Device layout: time on partitions ([128 t, 768 c] tiles), 32 tiles/core.
Pass 1: exp (Scalar) + egv (DVE) + R block-sums (PE) -> SBUF-resident eg/egv.
Level 2: PE transposes R -> 24 short DVE scans -> PE transposes back (bf16 G).
Pass 2: qlam mults (DVE), num/den assembled entirely in PSUM (PE), recip
(DVE), y mult (GpSimd/DVE alternating), bf16 DMA out.
Data-parallel over B across the 8 cores.
"""

import numpy as np
import ml_dtypes

import concourse.bacc as bacc
import concourse.bass as bass
import concourse.mybir as mybir
from concourse.bass_utils import run_bass_kernel_spmd
from concourse.tile import TileContext
from concourse.masks import make_identity

AluOp = mybir.AluOpType
AFT = mybir.ActivationFunctionType
F32 = mybir.dt.float32
BF16 = mybir.dt.bfloat16
F16 = mybir.dt.float16

B0, T0, C0 = 8, 4096, 768
NCORES = 8
P = 128
L = 32                 # subchunk length
SPT = P // L           # 4 subchunks per tile
NT = T0 // P           # 32 tiles
NS = T0 // L           # 128 subchunks
HB = 384               # psum half width (bank limit 512 f32)
NG = C0 // P           # 6 channel groups
HTILES = NT // 2       # 16 tiles per half


def _build_nc() -> bass.Bass:
    nc = bacc.Bacc()
    argT = nc.dram_tensor("argT", [T0, C0], F32, kind="ExternalInput")
    vT = nc.dram_tensor("vT", [T0, C0], BF16, kind="ExternalInput")
    qlm = nc.dram_tensor("qlm", [P, C0], BF16, kind="ExternalInput")
    trix = nc.dram_tensor("trix", [P, P], BF16, kind="ExternalInput")
    idm = nc.dram_tensor("idm", [P, P], BF16, kind="ExternalInput")
    # R-extraction lhsT: 8 per-tile-in-group variants of [128, 64], stacked
    rpa = nc.dram_tensor("rpa", [P, 512], BF16, kind="ExternalInput")
    rpb = nc.dram_tensor("rpb", [P, 512], BF16, kind="ExternalInput")
    # carry-pick lhsT: 16 per-tile variants of [128, 128], stacked
    pka = nc.dram_tensor("pka", [P, 16 * P], BF16, kind="ExternalInput")
    pkb = nc.dram_tensor("pkb", [P, 16 * P], BF16, kind="ExternalInput")
    lam32 = nc.dram_tensor("lam32", [P, NG], F32, kind="ExternalInput")
    lam16 = nc.dram_tensor("lam16", [P, NG], F32, kind="ExternalInput")
    yT = nc.dram_tensor("yT", [T0, C0], F16, kind="ExternalOutput")

    with TileContext(nc) as tc:
        with (
            tc.tile_pool(name="const", bufs=1) as cpool,
            tc.tile_pool(name="persist", bufs=1) as ppool,
            tc.tile_pool(name="work", bufs=3) as pool,
        ):
            qlmc = cpool.tile([P, C0], BF16)
            nc.sync.dma_start(qlmc[:], qlm[:])
            trixc = cpool.tile([P, P], BF16)
            nc.sync.dma_start(trixc[:], trix[:])
            idmc = cpool.tile([P, P], BF16)
            nc.sync.dma_start(idmc[:], idm[:])
            rpac = cpool.tile([P, 512], BF16)
            nc.sync.dma_start(rpac[:], rpa[:])
            rpbc = cpool.tile([P, 512], BF16)
            nc.sync.dma_start(rpbc[:], rpb[:])
            pkac = cpool.tile([P, 16 * P], BF16)
            nc.sync.dma_start(pkac[:], pka[:])
            pkbc = cpool.tile([P, 16 * P], BF16)
            nc.sync.dma_start(pkbc[:], pkb[:])
            lam32c = cpool.tile([P, NG], F32)
            nc.sync.dma_start(lam32c[:], lam32[:])
            lam16c = cpool.tile([P, NG], F32)
            nc.sync.dma_start(lam16c[:], lam16[:])
            idf = cpool.tile([P, P], F32)
            make_identity(nc, idf[:])

            egtiles = [ppool.tile([P, C0], BF16, name=f"egt_{t}")
                       for t in range(NT)]
            egvtiles = [ppool.tile([P, C0], BF16, name=f"egvt_{t}")
                        for t in range(NT)]
            # R rows: per tile 8 rows (a/b interleaved per subchunk), halves
            rab = ppool.tile([P, 2 * C0], F32)
            # transposed-back carries, bf16, rows = interleaved s2 per half
            gt = ppool.tile([P, 2 * C0], BF16)

            # level-2 state (persistent: scans chain across halves)
            gi = ppool.tile([P, NG * 256], F32, name="gi")
            mtas = [ppool.tile([P, 129], F32, name=f"mta_{g}") for g in range(NG)]
            mtbs = [ppool.tile([P, 129], F32, name=f"mtb_{g}") for g in range(NG)]
            lambs = [ppool.tile([P, 64], F32, name=f"lamb_{g}") for g in range(NG)]
            for g in range(NG):
                nc.vector.memset(lambs[g][:], 1.0)
                nc.vector.tensor_scalar_mul(lambs[g][:], lambs[g][:],
                                            lam32c[:, g : g + 1])
                nc.vector.memset(mtas[g][:, 0:1], 0.0)
                nc.vector.memset(mtbs[g][:, 0:1], 0.0)

            # PSUM pools open across the whole pipeline: 2 + 2 + 4 = 8 banks
            with (
                tc.tile_pool(name="psr", bufs=1, space="PSUM") as psr,
                tc.tile_pool(name="ps2", bufs=1, space="PSUM") as ps2,
            ):
                cur_pr = [None]

                def pass1_tile(t):
                    at = pool.tile([P, C0], F32, tag="at", bufs=3, name=f"at_{t}")
                    nc.sync.dma_start(at[:], argT[P * t : P * (t + 1), :])
                    vt = pool.tile([P, C0], BF16, tag="vt", bufs=3,
                                   name=f"vt_{t}")
                    nc.sync.dma_start(vt[:], vT[P * t : P * (t + 1), :])
                    eg = egtiles[t][:]
                    nc.scalar.activation(eg, at[:], AFT.Exp)
                    egv = egvtiles[t][:]
                    nc.vector.tensor_tensor(egv, eg, vt[:], op=AluOp.mult)
                    # R rows for groups of 8 tiles accumulate into one
                    # [64, HB] psum tile; per-tile lhsT variants write
                    # disjoint rows (accumulation adds zeros elsewhere).
                    h = t // HTILES
                    m8 = t % 8
                    grp = t // 8
                    if m8 == 0:
                        cur_pr[0] = psr.tile([P, 512], F32, tag="pr",
                                             bufs=1, name=f"pr_{grp}")
                    lcols = slice(64 * m8, 64 * (m8 + 1))
                    for hb in range(2):
                        cols = slice(hb * HB, (hb + 1) * HB)
                        out = cur_pr[0][64 * hb : 64 * (hb + 1), 0:HB]
                        nc.tensor.matmul(out, rpac[:, lcols], egv[:, cols],
                                         start=(m8 == 0), stop=False)
                    for hb in range(2):
                        cols = slice(hb * HB, (hb + 1) * HB)
                        out = cur_pr[0][64 * hb : 64 * (hb + 1), 0:HB]
                        nc.tensor.matmul(out, rpbc[:, lcols], eg[:, cols],
                                         start=False, stop=(m8 == 7))
                    if m8 == 7:
                        r0 = 64 * (grp % 2)
                        for hb in range(2):
                            nc.scalar.copy(
                                rab[r0 : r0 + 64,
                                    h * C0 + hb * HB : h * C0 + (hb + 1) * HB],
                                cur_pr[0][64 * hb : 64 * (hb + 1), 0:HB],
                            )

                def level2_half(h):
                    # scans read the transposed R directly from PSUM; the
                    # lam32 = lam16*lam16 scaling (lam32 itself underflows
                    # f32 for strongly negative w) is applied afterwards on
                    # the small G outputs, in two stages
                    hc = h * C0
                    for g in range(NG):
                        pt0 = psr.tile([P, 512], F32, tag="pr", bufs=1,
                                       name=f"pt_{h}_{g}")
                        nc.tensor.transpose(pt0[:, 0:P],
                                            rab[:, hc + g * P : hc + (g + 1) * P],
                                            idf[:])
                        # chained scans: a at even cols, b at odd cols
                        mta, mtb = mtas[g], mtbs[g]
                        c0s = 1 + 64 * h
                        init_a = 0.0 if h == 0 else mta[:, 64:65]
                        init_b = 0.0 if h == 0 else mtb[:, 64:65]
                        nc.vector.tensor_tensor_scan(
                            mta[:, c0s : c0s + 64], lambs[g][:],
                            pt0[:, 0:128:2], init_a,
                            op0=AluOp.mult, op1=AluOp.add)
                        nc.vector.tensor_tensor_scan(
                            mtb[:, c0s : c0s + 64], lambs[g][:],
                            pt0[:, 1:128:2], init_b,
                            op0=AluOp.mult, op1=AluOp.add)
                        # G_S = lam32 * (scan state at S-1), raw-R scan:
                        # apply lam16 twice while interleaving into gi
                        gsl = gi[:, 256 * g + 128 * h : 256 * g + 128 * (h + 1)]
                        nc.vector.tensor_scalar_mul(
                            gsl[:, 0:128:2], mtas[g][:, 64 * h : 64 * (h + 1)],
                            lam16c[:, g : g + 1])
                        nc.vector.tensor_scalar_mul(
                            gsl[:, 0:128:2], gsl[:, 0:128:2],
                            lam16c[:, g : g + 1])
                        nc.vector.tensor_scalar_mul(
                            gsl[:, 1:128:2], mtbs[g][:, 64 * h : 64 * (h + 1)],
                            lam16c[:, g : g + 1])
                        nc.vector.tensor_scalar_mul(
                            gsl[:, 1:128:2], gsl[:, 1:128:2],
                            lam16c[:, g : g + 1])
                        ptg = psr.tile([P, 512], F32, tag="pr", bufs=1,
                                       name=f"ptg_{h}_{g}")
                        nc.tensor.transpose(ptg[:, 0:P], gsl, idf[:])
                        nc.scalar.copy(gt[:, hc + g * P : hc + (g + 1) * P],
                                       ptg[:, 0:P])

                def pass2_tile(t):
                    eg = egtiles[t][:]
                    egv = egvtiles[t][:]
                    eg2 = pool.tile([P, C0], BF16, tag="eg2", bufs=2,
                                    name=f"eg2_{t}")
                    # on odd tiles DVE owns the y-mult; eg2 goes to gpsimd
                    eng2 = nc.gpsimd if t % 2 == 1 else nc.vector
                    eng2.tensor_tensor(eg2[:], eg, qlmc[:], op=AluOp.mult)
                    egvq = pool.tile([P, C0], BF16, tag="egvq", bufs=2,
                                     name=f"egvq_{t}")
                    nc.vector.tensor_tensor(egvq[:], egv, qlmc[:], op=AluOp.mult)
                    h = t // HTILES
                    m = t % HTILES
                    pcols = slice(m * P, (m + 1) * P)
                    yt = pool.tile([P, C0], F16, tag="yt", bufs=2, name=f"yt_{t}")
                    pns_ = [None, None]
                    recs = [None, None]
                    pds = [None, None]
                    for hb in range(2):
                        pnum = ps2.tile([P, 512], F32, tag=f"pn{hb}", bufs=2,
                                        name=f"pn_{t}_{hb}")
                        pns_[hb] = pnum[:, 0:HB]
                        pden = ps2.tile([P, 512], F32, tag=f"pd{hb}", bufs=1,
                                        name=f"pd_{t}_{hb}")
                        pds[hb] = pden[:, 0:HB]
                    # weight-reuse matmul order
                    for hb in range(2):
                        cols = slice(hb * HB, (hb + 1) * HB)
                        nc.tensor.matmul(pns_[hb], trixc[:], egv[:, cols],
                                         start=True, stop=False)
                        nc.tensor.matmul(pds[hb], trixc[:], eg[:, cols],
                                         start=True, stop=False)
                    for hb in range(2):
                        cols = slice(hb * HB, (hb + 1) * HB)
                        nc.tensor.matmul(pns_[hb], idmc[:], egvq[:, cols],
                                         start=False, stop=False)
                        nc.tensor.matmul(pds[hb], idmc[:], eg2[:, cols],
                                         start=False, stop=False)
                    for hb in range(2):
                        gcols = slice(h * C0 + hb * HB, h * C0 + (hb + 1) * HB)
                        nc.tensor.matmul(pns_[hb], pkac[:, pcols], gt[:, gcols],
                                         start=False, stop=True)
                    for hb in range(2):
                        gcols = slice(h * C0 + hb * HB, h * C0 + (hb + 1) * HB)
                        nc.tensor.matmul(pds[hb], pkbc[:, pcols], gt[:, gcols],
                                         start=False, stop=True)
                    for hb in range(2):
                        cols = slice(hb * HB, (hb + 1) * HB)
                        rec = pool.tile([P, HB], F32, tag=f"rec{hb}", bufs=2,
                                        name=f"rec_{t}_{hb}")
                        nc.vector.reciprocal_approx_fast(rec[:], pds[hb])
                        recs[hb] = rec
                        nc.vector.tensor_tensor(yt[:, cols], pns_[hb],
                                                recs[hb][:], op=AluOp.mult)
                    nc.scalar.dma_start(yT[P * t : P * (t + 1), :], yt[:])

                # ---- half-pipelined schedule ----
                for t in range(HTILES):
                    pass1_tile(t)
                level2_half(0)
                for i in range(HTILES - 2):
                    pass1_tile(HTILES + i)
                    pass2_tile(i)
                pass1_tile(NT - 2)
                pass1_tile(NT - 1)
                # emit level2(h1) before the last h0 pass2 tiles so its
                # transposes aren't queued behind them on the PE
                level2_half(1)
                pass2_tile(HTILES - 2)
                pass2_tile(HTILES - 1)
                for t in range(HTILES, NT):
                    pass2_tile(t)
    nc.finalize()
    return nc


_NC_CACHE: list = []


def _get_nc() -> bass.Bass:
    if not _NC_CACHE:
        _NC_CACHE.append(_build_nc())
    return _NC_CACHE[0]


def _host_consts(w: np.ndarray, u: np.ndarray):
    w64 = w.astype(np.float64)
    u64 = u.astype(np.float64)
    sig = np.maximum(w64, 0.0)
    lam32v = np.exp(np.minimum(w64, 0.0) * L).astype(np.float32)
    lam16v = np.exp(np.minimum(w64, 0.0) * (L // 2)).astype(np.float32)
    qlam = np.exp(u64 + w64).astype(np.float32)
    beta = (L - 1) / 2.0 * np.maximum(-w64, 0.0)

    qlm = np.ascontiguousarray(
        np.broadcast_to(qlam.astype(ml_dtypes.bfloat16), (P, C0))
    )
    j = np.arange(P)
    blk = j // L
    trix = ((blk[:, None] == blk[None, :]) & (j[:, None] < j[None, :])).astype(
        ml_dtypes.bfloat16
    )
    idm = np.eye(P, dtype=ml_dtypes.bfloat16)
    # R-extraction lhsT variants: m8-th variant is [128, 64] with ones at
    # [j, 8*m8 + 2*(j//L) (+1 for b)]
    rpa = np.zeros((P, 512), dtype=ml_dtypes.bfloat16)
    rpb = np.zeros((P, 512), dtype=ml_dtypes.bfloat16)
    for m8 in range(8):
        rcol = 64 * m8 + 8 * m8 + 2 * blk
        rpa[j, rcol] = 1
        rpb[j, rcol + 1] = 1
    # carry-pick lhsT variants: m-th is [128, 128] with pka[r, i] = 1 iff
    # r == 8*m + 2*(i//L) (+1 for b)
    pka = np.zeros((P, 16 * P), dtype=ml_dtypes.bfloat16)
    pkb = np.zeros((P, 16 * P), dtype=ml_dtypes.bfloat16)
    for m in range(16):
        rrow = 8 * m + 2 * blk
        pka[rrow, m * P + j] = 1
        pkb[rrow + 1, m * P + j] = 1
    lam32col = np.ascontiguousarray(lam32v.reshape(NG, P).T)
    lam16col = np.ascontiguousarray(lam16v.reshape(NG, P).T)

    t_idx = np.arange(T0)
    jmod = (t_idx % L).astype(np.float64)
    tau = (t_idx - jmod).astype(np.float64)
    argbase = (
        -jmod[:, None] * w64[None, :]
        - tau[:, None] * sig[None, :]
        - beta[None, :]
    )
    return qlm, trix, idm, rpa, rpb, pka, pkb, lam32col, lam16col, argbase


def kernel(B, T, C, w, u, k, v):
    B, T, C = int(B), int(T), int(C)
    assert (B, T, C) == (B0, T0, C0), f"compiled for {(B0, T0, C0)}, got {(B, T, C)}"
    w = np.asarray(w, dtype=np.float32)
    u = np.asarray(u, dtype=np.float32)
    k = np.asarray(k, dtype=np.float32)
    v = np.asarray(v, dtype=np.float32)

    qlm, trix, idm, rpa, rpb, pka, pkb, lam32col, lam16col, argbase = _host_consts(
        w, u
    )
    in_maps = []
    for b in range(NCORES):
        arg_b = (k[b].astype(np.float64) + argbase).astype(np.float32)
        in_maps.append(
            {
                "argT": arg_b,
                "vT": v[b].astype(ml_dtypes.bfloat16),
                "qlm": qlm,
                "trix": trix,
                "idm": idm,
                "rpa": rpa,
                "rpb": rpb,
                "pka": pka,
                "pkb": pkb,
                "lam32": lam32col,
                "lam16": lam16col,
            }
        )

    res = run_bass_kernel_spmd(_get_nc(), in_maps, list(range(NCORES)))
    out = np.stack(
        [res.results[i]["yT"].astype(np.float32) for i in range(NCORES)], axis=0
    )
    return np.ascontiguousarray(out)
